# revision 1
# baseline (speedup 1.0000x reference)
"""Trainium2 Bass kernel for CustomTriangleMultiplicationOutgoing.

Reference computation (B=1, N=384, D=C=128):
    z_norm = LN(z) * g + b                        # over D
    left   = (z_norm@Wa + ba) * sigmoid(z_norm@Wga + bga) * mask
    right  = (z_norm@Wb + bb) * sigmoid(z_norm@Wgb + bgb) * mask
    z_out[i,j,c] = sum_k left[i,k,c] * right[j,k,c]
    z_out  = LN(z_out) * g_out + b_out            # over C
    out    = (z_out@Wo + bo) * sigmoid(z_norm@Wgo + bgo)

Sharding: 1D over the first N (i) axis, 48 rows per core.  Each core
computes its row-shard's projections locally (left + out-gate stay in
SBUF in bf16), the gated right projection is AllGathered in bf16 across
the 8 cores in 4 c-range chunks (pipelined against the einsum), the
einsum runs on the tensor engine with k on partitions, and the final
LN + Wo + gate is fused on the way out.

Engine discipline: ACT runs a single activation function per phase
(batched Sqrt for phase-1 LN, Sigmoid for gates, Sqrt again in phase 3)
to avoid 1.3us activation-table reloads; matmul operands are bf16 so
FWL fast weight loads engage.
"""

import numpy as np
import ml_dtypes

import concourse.bass as bass
import concourse.mybir as mybir
import concourse.tile as tile
from concourse import bacc
from concourse.masks import make_identity
from concourse.bass_utils import run_bass_kernel_spmd

F32 = mybir.dt.float32
BF16 = mybir.dt.bfloat16
EPS = 1e-5

B = 1
N_FULL = 384
D = 128
C = 128
W = 8  # cores
P = 128


def bcast_part(ap, parts):
    """Broadcast a [1, ...] AP across `parts` partitions (partition step 0)."""
    return bass.AP(tensor=ap.tensor, offset=ap.offset, ap=[[0, parts]] + ap.ap[1:])


def build_nc(n=N_FULL, with_bias=False, with_mask=False, nq=4, cb=8):
    """Build the SPMD Bass program (same program on all 8 cores)."""
    assert n % P == 0 and n % W == 0
    SH = n // W          # rows of i per core
    KC = n // P          # 128-wide chunks of k
    NT = SH * n // P     # phase-1 tiles per core (= SH*KC)
    CQ = C // nq         # c per AllGather chunk
    assert CQ % cb == 0

    nc = bacc.Bacc(None, num_devices=W)

    z_sh = nc.declare_dram_parameter("z_sh", [SH * n, D], BF16, isOutput=False)
    wcat = nc.declare_dram_parameter("wcat", [D, 4 * C], BF16, isOutput=False)
    wgo = nc.declare_dram_parameter("wgo", [D, D], BF16, isOutput=False)
    wo = nc.declare_dram_parameter("wo", [C, D + 1], BF16, isOutput=False)
    wosum = nc.declare_dram_parameter("wosum", [1, D], F32, isOutput=False)
    if with_bias:
        bcat = nc.declare_dram_parameter("bcat", [1, 4 * C], F32, isOutput=False)
        bgo_p = nc.declare_dram_parameter("bgo", [1, D], F32, isOutput=False)
        bo_p = nc.declare_dram_parameter("bo", [1, D], F32, isOutput=False)
    if with_mask:
        mask_sh = nc.declare_dram_parameter("mask_sh", [SH * n], F32, isOutput=False)
    out_sh = nc.declare_dram_parameter("out_sh", [SH * n, D], F32, isOutput=True)

    # internal DRAM
    right_q = [nc.dram_tensor(f"right_{q}", [KC, P, CQ, SH], BF16) for q in range(nq)]
    gath_q = [
        nc.dram_tensor(f"gath_{q}", [W, KC, P, CQ, SH], BF16, addr_space="Shared")
        for q in range(nq)
    ]
    zout = nc.dram_tensor("zout", [C, SH, n], BF16)  # c-major einsum result

    with tile.TileContext(nc) as tc:
        with tc.tile_pool(name="singles", bufs=1) as singles:
            ident = singles.tile([P, P], BF16)
            make_identity(nc, ident)
            wcat_sb = singles.tile([D, 4 * C], BF16)
            nc.sync.dma_start(wcat_sb, wcat[:])
            wgo_sb = singles.tile([D, D], BF16)
            nc.sync.dma_start(wgo_sb, wgo[:])
            wo_sb = singles.tile([C, D + 1], BF16)
            nc.sync.dma_start(wo_sb, wo[:])
            wosum_b = singles.tile([P, D], F32)
            nc.sync.dma_start(wosum_b, bcast_part(wosum[:], P))
            ones_bf = singles.tile([C, 1], BF16)
            nc.vector.memset(ones_bf, 1.0)
            eps_sb = singles.tile([P, 1], F32)
            nc.vector.memset(eps_sb, EPS)
            if with_bias:
                bcat_sb = singles.tile([P, 4 * C], F32)
                nc.sync.dma_start(bcat_sb, bcast_part(bcat[:], P))
                bgo_sb = singles.tile([P, D], F32)
                nc.sync.dma_start(bgo_sb, bcast_part(bgo_p[:], P))
                bo_sb = singles.tile([P, D], F32)
                nc.sync.dma_start(bo_sb, bcast_part(bo_p[:], P))
            if with_mask:
                mask_sb = singles.tile([P, NT], F32)
                nc.sync.dma_start(
                    mask_sb, mask_sh[:].rearrange("(t p) -> p t", p=P)
                )

            # persistent bf16 stores
            L_sb = singles.tile([P, KC, SH, C], BF16)   # left:  [k, kc, i_loc, c]
            gg_sb = singles.tile([P, NT, D], BF16)      # out-gate per row tile

            # ---------------- phase 1: LN + projections ----------------
            with (
                tc.tile_pool(name="p1_z", bufs=1) as zpool,
                tc.tile_pool(name="p1_temps", bufs=4) as temps,
                tc.tile_pool(name="p1_stats", bufs=1) as spool,
                tc.tile_pool(name="p1_psum", bufs=2, space="PSUM") as psum1,
                tc.tile_pool(name="p1_rstage", bufs=1) as rstage,
            ):
                zbuf = zpool.tile([P, NT, D], BF16)
                nc.sync.dma_start(zbuf, z_sh[:].rearrange("(t p) d -> p t d", p=P))
                # batched LN stats: one Sqrt for all tiles
                mv_all = spool.tile([P, NT, 2], F32)
                for t in range(NT):
                    stats = temps.tile([P, 6], F32, tag="stats")
                    nc.vector.bn_stats(stats, zbuf[:, t, :])
                    nc.vector.bn_aggr(mv_all[:, t, :], stats)
                rstd_all = spool.tile([P, NT], F32)
                nc.scalar.activation(
                    rstd_all, mv_all[:, :, 1],
                    mybir.ActivationFunctionType.Sqrt, bias=eps_sb,
                )
                nc.vector.reciprocal(rstd_all, rstd_all)

                R_stage = rstage.tile([P, KC, C, SH], BF16)  # right: [k, kc, c, j_loc]
                for t in range(NT):
                    i_loc = t // KC
                    kc = t % KC
                    xn = temps.tile([P, D], BF16, tag="xn")
                    nc.vector.tensor_scalar(
                        xn,
                        zbuf[:, t, :],
                        scalar1=mv_all[:, t, 0:1],
                        scalar2=rstd_all[:, t : t + 1],
                        op0=mybir.AluOpType.subtract,
                        op1=mybir.AluOpType.mult,
                    )
                    # transpose z_norm tile -> T [D, rows]
                    pt = psum1.tile([P, P], BF16, tag="pt")
                    nc.tensor.transpose(pt, xn, ident)
                    T = temps.tile([P, P], BF16, tag="T")
                    nc.vector.tensor_copy(T, pt)
                    # projections: [rows, a|ga|b|gb] and [rows, go]
                    p1 = psum1.tile([P, 4 * C], F32, tag="p1")
                    nc.tensor.matmul(p1, lhsT=T, rhs=wcat_sb, start=True, stop=True)
                    p2 = psum1.tile([P, D], F32, tag="p2")
                    nc.tensor.matmul(p2, lhsT=T, rhs=wgo_sb, start=True, stop=True)
                    if with_bias:
                        nc.vector.tensor_tensor(p1, p1, bcat_sb, mybir.AluOpType.add)
                        nc.vector.tensor_tensor(p2, p2, bgo_sb, mybir.AluOpType.add)
                    # gates (ACT: Sigmoid only)
                    sga = temps.tile([P, C], F32, tag="sga")
                    nc.scalar.activation(
                        sga, p1[:, C : 2 * C], mybir.ActivationFunctionType.Sigmoid
                    )
                    sgb = temps.tile([P, C], F32, tag="sgb")
                    nc.scalar.activation(
                        sgb, p1[:, 3 * C : 4 * C], mybir.ActivationFunctionType.Sigmoid
                    )
                    if with_mask:
                        nc.gpsimd.tensor_scalar_mul(sga, sga, mask_sb[:, t : t + 1])
                        nc.gpsimd.tensor_scalar_mul(sgb, sgb, mask_sb[:, t : t + 1])
                    # out-gate (sigmoid straight to bf16 store)
                    nc.scalar.activation(
                        gg_sb[:, t, :], p2, mybir.ActivationFunctionType.Sigmoid
                    )
                    # gated products into einsum-layout stores
                    nc.vector.tensor_tensor(
                        L_sb[:, kc, i_loc, :], p1[:, 0:C], sga, mybir.AluOpType.mult
                    )
                    nc.vector.tensor_tensor(
                        R_stage[:, kc, :, i_loc],
                        p1[:, 2 * C : 3 * C],
                        sgb,
                        mybir.AluOpType.mult,
                    )
                # write right shard to DRAM AllGather inputs
                for q in range(nq):
                    for kc in range(KC):
                        nc.sync.dma_start(
                            right_q[q][kc],
                            R_stage[:, kc, q * CQ : (q + 1) * CQ, :],
                        )

            # ---------------- phase 1.5: AllGather right ----------------
            for q in range(nq):
                nc.gpsimd.collective_compute(
                    "AllGather",
                    mybir.AluOpType.bypass,
                    replica_groups=[list(range(W))],
                    ins=[right_q[q][:]],
                    outs=[gath_q[q][:]],
                )

            # ---------------- phase 2: einsum ----------------
            with (
                tc.tile_pool(name="p2_r", bufs=2) as rpool,
                tc.tile_pool(name="p2_st", bufs=4) as stpool,
                tc.tile_pool(name="p2_psum", bufs=8, space="PSUM") as psum2,
            ):
                for q in range(nq):
                    for cbi in range(CQ // cb):
                        R_blk = rpool.tile([P, KC, W, cb, SH], BF16, tag="rblk")
                        for kc in range(KC):
                            for m in range(W):
                                nc.sync.dma_start(
                                    R_blk[:, kc, m],
                                    gath_q[q][m, kc, :, cbi * cb : (cbi + 1) * cb, :],
                                )
                        for c_ in range(cb):
                            c_glob = q * CQ + cbi * cb + c_
                            ps = psum2.tile([SH, n], F32, tag="ps")
                            for kc in range(KC):
                                nc.tensor.matmul(
                                    ps,
                                    lhsT=L_sb[:, kc, :, c_glob],
                                    rhs=R_blk[:, kc, :, c_, :],
                                    start=(kc == 0),
                                    stop=(kc == KC - 1),
                                )
                            st = stpool.tile([SH, n], BF16, tag="st")
                            if c_ % 2 == 0:
                                nc.vector.tensor_copy(st, ps)
                            else:
                                nc.scalar.copy(st, ps)
                            nc.sync.dma_start(zout[c_glob], st)

            # ---------------- phase 3: LN(z_out) @ Wo * gate ----------------
            zout_flat = zout[:].rearrange("c i j -> c (i j)")
            with (
                tc.tile_pool(name="p3_temps", bufs=4) as t3,
                tc.tile_pool(name="p3_psum", bufs=2, space="PSUM") as psum3,
            ):
                for rt in range(NT):
                    zt = t3.tile([C, P], BF16, tag="zt")
                    nc.sync.dma_start(zt, zout_flat[:, P * rt : P * (rt + 1)])
                    sq = t3.tile([C, P], BF16, tag="sq")
                    nc.vector.tensor_tensor(sq, zt, zt, mybir.AluOpType.mult)
                    # pr[:, 0:D] = Z.T @ Wo' ; pr[:, D] = per-row sum of Z (ones col)
                    pr = psum3.tile([P, D + 1], F32, tag="pr")
                    nc.tensor.matmul(pr, lhsT=zt, rhs=wo_sb, start=True, stop=True)
                    ss = psum3.tile([P, 1], F32, tag="ss")
                    nc.tensor.matmul(ss, lhsT=sq, rhs=ones_bf, start=True, stop=True)
                    mean = t3.tile([P, 1], F32, tag="mean")
                    nc.vector.tensor_scalar_mul(mean, pr[:, D : D + 1], 1.0 / C)
                    msq = t3.tile([P, 1], F32, tag="msq")
                    nc.vector.tensor_scalar_mul(msq, ss, 1.0 / C)
                    var = t3.tile([P, 1], F32, tag="var")
                    nc.vector.tensor_tensor(var, mean, mean, mybir.AluOpType.mult)
                    nc.vector.tensor_tensor(var, msq, var, mybir.AluOpType.subtract)
                    rstd3 = t3.tile([P, 1], F32, tag="rstd3")
                    nc.scalar.activation(
                        rstd3, var, mybir.ActivationFunctionType.Sqrt, bias=eps_sb
                    )
                    nc.vector.reciprocal(rstd3, rstd3)
                    sc = t3.tile([P, 1], F32, tag="sc")
                    nc.vector.tensor_tensor(sc, mean, rstd3, mybir.AluOpType.mult)
                    # out = rstd*(Z.T@Wo) - (rstd*mean)*colsum(Wo)  [+ bo]
                    corr = t3.tile([P, D], F32, tag="corr")
                    nc.vector.tensor_scalar_mul(corr, wosum_b, sc)
                    po = t3.tile([P, D], F32, tag="po")
                    nc.vector.tensor_scalar_mul(po, pr[:, 0:D], rstd3)
                    nc.vector.tensor_tensor(po, po, corr, mybir.AluOpType.subtract)
                    if with_bias:
                        nc.vector.tensor_tensor(po, po, bo_sb, mybir.AluOpType.add)
                    ot = t3.tile([P, D], F32, tag="ot")
                    nc.vector.tensor_tensor(
                        ot, po, gg_sb[:, rt, :], mybir.AluOpType.mult
                    )
                    nc.sync.dma_start(out_sh[P * rt : P * (rt + 1), :], ot)

    nc.compile()
    return nc


_CACHE = {}


def _get_nc(n, with_bias, with_mask):
    key = (n, with_bias, with_mask)
    if key not in _CACHE:
        _CACHE[key] = build_nc(n=n, with_bias=with_bias, with_mask=with_mask)
    return _CACHE[key]


def prepare_host(z, mask, norm_g, norm_b, norm_out_g, norm_out_b,
                 Wa, ba, Wb, bb, Wga, bga, Wgb, bgb, Wo, bo, Wgo, bgo, n=N_FULL):
    """Fold norm affines into weights; build per-core input maps."""
    f = np.asarray
    z = f(z, dtype=np.float32)
    mask = f(mask, dtype=np.float32)
    g = f(norm_g, np.float32)
    b = f(norm_b, np.float32)
    go = f(norm_out_g, np.float32)
    bo_n = f(norm_out_b, np.float32)

    def fold(Wm, bias):
        Wm = f(Wm, np.float32)
        bias = f(bias, np.float32)
        return g[:, None] * Wm, bias + b @ Wm

    Wa_, ba_ = fold(Wa, ba)
    Wga_, bga_ = fold(Wga, bga)
    Wb_, bb_ = fold(Wb, bb)
    Wgb_, bgb_ = fold(Wgb, bgb)
    Wgo_, bgo_ = fold(Wgo, bgo)
    Wo_ = go[:, None] * f(Wo, np.float32)
    bo_ = f(bo, np.float32) + bo_n @ f(Wo, np.float32)

    bf = ml_dtypes.bfloat16
    wcat = np.concatenate([Wa_, Wga_, Wb_, Wgb_], axis=1).astype(bf)
    woa = np.concatenate([Wo_, np.ones((C, 1), np.float32)], axis=1).astype(bf)
    wosum_h = Wo_.sum(axis=0)[None, :].astype(np.float32)
    bcat = np.concatenate([ba_, bga_, bb_, bgb_])[None, :].astype(np.float32)

    with_bias = bool(np.any(bcat) or np.any(bgo_) or np.any(bo_))
    with_mask = not bool(np.all(mask == 1.0))

    SH = n // W
    in_maps = []
    for m in range(W):
        im = {
            "z_sh": np.ascontiguousarray(
                z[0, SH * m : SH * (m + 1)].reshape(SH * n, D)
            ).astype(bf),
            "wcat": wcat,
            "wgo": np.ascontiguousarray(Wgo_).astype(bf),
            "wo": woa,
            "wosum": wosum_h,
        }
        if with_bias:
            im["bcat"] = bcat
            im["bgo"] = bgo_[None, :].astype(np.float32)
            im["bo"] = bo_[None, :].astype(np.float32)
        if with_mask:
            im["mask_sh"] = np.ascontiguousarray(
                mask[0, SH * m : SH * (m + 1)].reshape(SH * n)
            )
        in_maps.append(im)
    return in_maps, with_bias, with_mask


def kernel(**inputs):
    n = inputs["z"].shape[1]
    in_maps, with_bias, with_mask = prepare_host(**inputs, n=n)
    nc = _get_nc(n, with_bias, with_mask)
    res = run_bass_kernel_spmd(nc, in_maps, list(range(W)))
    SH = n // W
    parts = [res.results[m]["out_sh"].reshape(SH, n, D) for m in range(W)]
    out = np.concatenate(parts, axis=0)[None]  # [1, n, n, D]
    return out.astype(np.float32)



# revision 8
# speedup vs baseline: 1.7988x; 1.7988x over previous
"""Trainium2 Bass kernel for CustomTriangleMultiplicationOutgoing.

Reference computation (B=1, N=384, D=C=128):
    z_norm = LN(z) * g + b                        # over D
    left   = (z_norm@Wa + ba) * sigmoid(z_norm@Wga + bga) * mask
    right  = (z_norm@Wb + bb) * sigmoid(z_norm@Wgb + bgb) * mask
    z_out[i,j,c] = sum_k left[i,k,c] * right[j,k,c]
    z_out  = LN(z_out) * g_out + b_out            # over C
    out    = (z_out@Wo + bo) * sigmoid(z_norm@Wgo + bgo)

Key identity exploited: row-wise LN commutes with the projection,
    LN(z) @ (g .* W) = (z * rstd) @ ((I - 11^T/D)(g .* W))
so the host passes zs = (z * rstd)^T in bf16 and centered/affine-folded
weights; the device does plain matmuls with NO LN work in phase 1.
Phase 3's LN over C uses the same centering on Wo; its rstd is computed
on-device from column sums of z_out (two tiny matmuls per tile against
a ones vector, batched rsqrt).

Sharding: 1D over the first N (i) axis, 48 rows per core.  Pass A
computes only the gated right projection so the 4 c-chunked AllGathers
(bf16) start ~50us in; pass B (left + out-gate) and phase 2 (einsum,
k on partitions) hide under the collectives.  Phase 3 reads z_out back
c-major in 4 large partition-contiguous DMAs and keeps everything else
SBUF-resident.
"""

import numpy as np
import ml_dtypes

import concourse.bass as bass
import concourse.mybir as mybir
import concourse.tile as tile
from concourse import bacc
from concourse.bass_utils import run_bass_kernel_spmd

F32 = mybir.dt.float32
BF16 = mybir.dt.bfloat16
EPS = 1e-5

B = 1
N_FULL = 384
D = 128
C = 128
W = 8  # cores
P = 128


def bcast_part(ap, parts):
    """Broadcast a [1, ...] AP across `parts` partitions (partition step 0)."""
    return bass.AP(tensor=ap.tensor, offset=ap.offset, ap=[[0, parts]] + ap.ap[1:])


def build_nc(n=N_FULL, with_bias=False, with_mask=False, nq=4):
    """Build the SPMD Bass program (same program on all 8 cores)."""
    assert n % P == 0 and n % W == 0
    SH = n // W          # rows of i per core
    KC = n // P          # 128-wide chunks of k
    NT = SH * n // P     # 128-row tiles per core (= SH*KC)
    CQ = C // nq         # c per AllGather chunk
    CQH = CQ // 2        # c per phase-2 half-load

    nc = bacc.Bacc(None, num_devices=W)

    zs = nc.declare_dram_parameter("zs", [P, NT * P], BF16, isOutput=False)
    wbg = nc.declare_dram_parameter("wbg", [D, 2 * C], BF16, isOutput=False)
    wago = nc.declare_dram_parameter("wago", [D, 2 * C + D], BF16, isOutput=False)
    wo = nc.declare_dram_parameter("wo", [C, D], BF16, isOutput=False)
    if with_bias:
        bbg_p = nc.declare_dram_parameter("bbg", [1, 2 * C], F32, isOutput=False)
        bago_p = nc.declare_dram_parameter("bago", [1, 2 * C + D], F32, isOutput=False)
        bo_p = nc.declare_dram_parameter("bo", [1, D], F32, isOutput=False)
    if with_mask:
        mask_sh = nc.declare_dram_parameter("mask_sh", [P, NT], F32, isOutput=False)
    out_sh = nc.declare_dram_parameter("out_sh", [P, NT, D], F32, isOutput=True)

    # internal DRAM
    right_q = [nc.dram_tensor(f"right_{q}", [P, KC, CQ, SH], BF16) for q in range(nq)]
    gath_q = [
        nc.dram_tensor(f"gath_{q}", [W, P, KC, CQ, SH], BF16, addr_space="Shared")
        for q in range(nq)
    ]
    zout = nc.dram_tensor("zout", [C, SH * n], BF16)  # c-major einsum result

    with tile.TileContext(nc) as tc:
        with tc.tile_pool(name="singles", bufs=1) as singles:
            wbg_sb = singles.tile([D, 2 * C], BF16)
            nc.sync.dma_start(wbg_sb, wbg[:])
            wago_sb = singles.tile([D, 2 * C + D], BF16)
            nc.sync.dma_start(wago_sb, wago[:])
            wo_sb = singles.tile([C, D], BF16)
            nc.sync.dma_start(wo_sb, wo[:])
            ones_bf = singles.tile([P, 1], BF16)
            nc.vector.memset(ones_bf, 1.0)
            eps_sb = singles.tile([P, 1], F32)
            nc.vector.memset(eps_sb, EPS)
            if with_bias:
                bbg_sb = singles.tile([P, 2 * C], F32)
                nc.sync.dma_start(bbg_sb, bcast_part(bbg_p[:], P))
                bago_sb = singles.tile([P, 2 * C + D], F32)
                nc.sync.dma_start(bago_sb, bcast_part(bago_p[:], P))
                bo_sb = singles.tile([P, D], F32)
                nc.sync.dma_start(bo_sb, bcast_part(bo_p[:], P))
            if with_mask:
                mask_sb = singles.tile([P, NT], F32)
                nc.sync.dma_start(mask_sb, mask_sh[:])

            # persistent bf16 stores
            gg_sb = singles.tile([P, NT, D], BF16)      # out-gate per row tile
            zt_all = singles.tile([C, NT * P], BF16)    # z_out, c on partitions

            lpool = tc.alloc_tile_pool(name="lpool", bufs=1)
            L_sb = lpool.tile([P, KC, SH, C], BF16)     # left: [k, kc, i_loc, c]

            p1pool = tc.alloc_tile_pool(name="p1", bufs=1)
            # phase-1 inputs (freed after pass B)
            zs_sb = p1pool.tile([P, NT * P], BF16)
            for ch in range(4):
                w4 = NT * P // 4
                nc.sync.dma_start(
                    zs_sb[:, ch * w4 : (ch + 1) * w4],
                    zs[:, ch * w4 : (ch + 1) * w4],
                )
            R_stage = p1pool.tile([P, KC, C, SH], BF16)  # right: [k, kc, c, j_loc]

            # ---------------- pass A: right projection ----------------
            with (
                tc.tile_pool(name="pA_tmp", bufs=4) as tmpsA,
                tc.tile_pool(name="pA_psum", bufs=4, space="PSUM") as psumA,
            ):
                for t in range(NT):
                    i_loc = t // KC
                    kc = t % KC
                    ps = psumA.tile([P, 2 * C], F32, tag="psA")
                    nc.tensor.matmul(
                        ps, lhsT=zs_sb[:, t * P : (t + 1) * P], rhs=wbg_sb,
                        start=True, stop=True,
                    )
                    if with_bias:
                        nc.vector.tensor_tensor(ps, ps, bbg_sb, mybir.AluOpType.add)
                    sgb = tmpsA.tile([P, C], F32, tag="sgb")
                    nc.scalar.activation(
                        sgb, ps[:, C : 2 * C], mybir.ActivationFunctionType.Sigmoid
                    )
                    if with_mask:
                        nc.gpsimd.tensor_scalar_mul(
                            sgb, sgb, mask_sb[:, t : t + 1]
                        )
                    nc.vector.tensor_tensor(
                        R_stage[:, kc, :, i_loc], ps[:, 0:C], sgb,
                        mybir.AluOpType.mult,
                    )
                for q in range(nq):
                    nc.sync.dma_start(
                        right_q[q][:], R_stage[:, :, q * CQ : (q + 1) * CQ, :]
                    )

            # ---------------- AllGather right (c-chunked) ----------------
            for q in range(nq):
                nc.gpsimd.collective_compute(
                    "AllGather",
                    mybir.AluOpType.bypass,
                    replica_groups=[list(range(W))],
                    ins=[right_q[q][:]],
                    outs=[gath_q[q][:]],
                )

            # ---------------- pass B: left + out-gate ----------------
            with (
                tc.tile_pool(name="pB_tmp", bufs=4) as tmpsB,
                tc.tile_pool(name="pB_psum", bufs=4, space="PSUM") as psumB,
            ):
                for t in range(NT):
                    i_loc = t // KC
                    kc = t % KC
                    ps = psumB.tile([P, 2 * C + D], F32, tag="psB")
                    nc.tensor.matmul(
                        ps, lhsT=zs_sb[:, t * P : (t + 1) * P], rhs=wago_sb,
                        start=True, stop=True,
                    )
                    if with_bias:
                        nc.vector.tensor_tensor(ps, ps, bago_sb, mybir.AluOpType.add)
                    sga = tmpsB.tile([P, C], F32, tag="sga")
                    nc.scalar.activation(
                        sga, ps[:, C : 2 * C], mybir.ActivationFunctionType.Sigmoid
                    )
                    nc.scalar.activation(
                        gg_sb[:, t, :], ps[:, 2 * C :],
                        mybir.ActivationFunctionType.Sigmoid,
                    )
                    if with_mask:
                        nc.gpsimd.tensor_scalar_mul(
                            sga, sga, mask_sb[:, t : t + 1]
                        )
                    nc.vector.tensor_tensor(
                        L_sb[:, kc, i_loc, :], ps[:, 0:C], sga,
                        mybir.AluOpType.mult,
                    )

            p1pool.release()  # zs, R_stage freed

            # ---------------- phase 2: einsum ----------------
            with (
                tc.tile_pool(name="p2_r", bufs=2) as rpool,
                tc.tile_pool(name="p2_st", bufs=3) as stpool,
                tc.tile_pool(name="p2_psum", bufs=6, space="PSUM") as psum2,
            ):
                for q in range(nq):
                    for h in range(2):
                        Rh = rpool.tile([P, KC, W, CQH, SH], BF16, tag="rh")
                        for m in range(W):
                            nc.sync.dma_start(
                                Rh[:, :, m],
                                gath_q[q][m, :, :, h * CQH : (h + 1) * CQH, :],
                            )
                        for c4 in range(CQH // 4):
                            stb = stpool.tile([SH, 4, n], BF16, tag="stb")
                            for c_ in range(4):
                                cl = c4 * 4 + c_
                                c_glob = q * CQ + h * CQH + cl
                                ps = psum2.tile([SH, n], F32, tag="ps")
                                for kc in range(KC):
                                    nc.tensor.matmul(
                                        ps,
                                        lhsT=L_sb[:, kc, :, c_glob],
                                        rhs=Rh[:, kc, :, cl, :],
                                        start=(kc == 0),
                                        stop=(kc == KC - 1),
                                    )
                                if c_ % 2 == 0:
                                    nc.vector.tensor_copy(stb[:, c_, :], ps)
                                else:
                                    nc.scalar.copy(stb[:, c_, :], ps)
                            c0 = q * CQ + h * CQH + c4 * 4
                            nc.sync.dma_start(
                                zout[c0 : c0 + 4].rearrange(
                                    "c (i j) -> i c j", i=SH
                                ),
                                stb,
                            )
                    # z_out c-rows for this chunk are final: fetch to SBUF
                    nc.sync.dma_start(
                        zt_all[q * CQ : (q + 1) * CQ, :],
                        zout[q * CQ : (q + 1) * CQ, :],
                    )

            lpool.release()  # L_sb freed

            # ---------------- phase 3: LN(z_out) @ Wo * gate ----------------
            with (
                tc.tile_pool(name="p3_tmp", bufs=4) as t3,
                tc.tile_pool(name="p3_big", bufs=1) as big3,
                tc.tile_pool(name="p3_sps", bufs=1, space="PSUM") as spsum,
                tc.tile_pool(name="p3_psum", bufs=4, space="PSUM") as psum3,
            ):
                stats_ps = spsum.tile([P, 2 * NT], F32)
                for t in range(NT):
                    zt = zt_all[:, t * P : (t + 1) * P]
                    sqv = t3.tile([C, P], BF16, tag="sqv")
                    nc.vector.tensor_tensor(sqv, zt, zt, mybir.AluOpType.mult)
                    nc.tensor.matmul(
                        stats_ps[:, 2 * t : 2 * t + 1], lhsT=zt, rhs=ones_bf,
                        start=True, stop=True,
                    )
                    nc.tensor.matmul(
                        stats_ps[:, 2 * t + 1 : 2 * t + 2], lhsT=sqv, rhs=ones_bf,
                        start=True, stop=True,
                    )
                stats = big3.tile([P, 2 * NT], F32)
                nc.vector.tensor_copy(stats, stats_ps)
                S = stats[:].rearrange("p (t two) -> p two t", two=2)[:, 0, :]
                SQ = stats[:].rearrange("p (t two) -> p two t", two=2)[:, 1, :]
                mean = big3.tile([P, NT], F32)
                nc.vector.tensor_scalar_mul(mean, S, 1.0 / C)
                msq = big3.tile([P, NT], F32)
                nc.vector.tensor_scalar_mul(msq, SQ, 1.0 / C)
                var = big3.tile([P, NT], F32)
                nc.vector.tensor_tensor(var, mean, mean, mybir.AluOpType.mult)
                nc.vector.tensor_tensor(var, msq, var, mybir.AluOpType.subtract)
                rstd = big3.tile([P, NT], F32)
                nc.scalar.activation(
                    rstd, var, mybir.ActivationFunctionType.Sqrt, bias=eps_sb
                )
                nc.vector.reciprocal(rstd, rstd)

                ot_sb = big3.tile([P, NT, D], F32)
                for t in range(NT):
                    pr = psum3.tile([P, D], F32, tag="pr")
                    nc.tensor.matmul(
                        pr, lhsT=zt_all[:, t * P : (t + 1) * P], rhs=wo_sb,
                        start=True, stop=True,
                    )
                    if with_bias:
                        po = t3.tile([P, D], F32, tag="po")
                        nc.vector.tensor_scalar_mul(po, pr, rstd[:, t : t + 1])
                        nc.vector.tensor_tensor(po, po, bo_sb, mybir.AluOpType.add)
                        nc.vector.tensor_tensor(
                            ot_sb[:, t, :], po, gg_sb[:, t, :], mybir.AluOpType.mult
                        )
                    else:
                        nc.vector.scalar_tensor_tensor(
                            ot_sb[:, t, :], pr, rstd[:, t : t + 1],
                            gg_sb[:, t, :],
                            mybir.AluOpType.mult, mybir.AluOpType.mult,
                        )
                for ch in range(4):
                    t0 = NT // 4 * ch
                    t1 = NT // 4 * (ch + 1)
                    nc.sync.dma_start(
                        out_sh[:, t0:t1, :], ot_sb[:, t0:t1, :]
                    )

    nc.compile()
    return nc


_CACHE = {}


def _get_nc(n, with_bias, with_mask):
    key = (n, with_bias, with_mask)
    if key not in _CACHE:
        _CACHE[key] = build_nc(n=n, with_bias=with_bias, with_mask=with_mask)
    return _CACHE[key]


def prepare_host(z, mask, norm_g, norm_b, norm_out_g, norm_out_b,
                 Wa, ba, Wb, bb, Wga, bga, Wgb, bgb, Wo, bo, Wgo, bgo, n=N_FULL):
    """Fold norm affines + centering into weights; pre-normalize z rows."""
    f = np.asarray
    z = f(z, dtype=np.float32)
    mask = f(mask, dtype=np.float32)
    g = f(norm_g, np.float32)
    b = f(norm_b, np.float32)
    go = f(norm_out_g, np.float32)
    bo_n = f(norm_out_b, np.float32)

    # LN(z) = (z - m) * r * g + b ; proj: LN(z) @ W + bias
    #       = (z*r) @ Wcen + (b @ W + bias)   with Wcen = (I - J/D)(g .* W)
    def fold(Wm, bias):
        Wm = f(Wm, np.float32)
        Wg = g[:, None] * Wm
        Wcen = Wg - np.mean(Wg, axis=0, keepdims=True)
        return Wcen, f(bias, np.float32) + b @ Wm

    Wa_, ba_ = fold(Wa, ba)
    Wga_, bga_ = fold(Wga, bga)
    Wb_, bb_ = fold(Wb, bb)
    Wgb_, bgb_ = fold(Wgb, bgb)
    Wgo_, bgo_ = fold(Wgo, bgo)
    # phase 3: LN_out(zout) @ Wo + bo = (zout*r3) @ Wocen + (bo_n @ Wo + bo)
    Wo32 = f(Wo, np.float32)
    Wog = go[:, None] * Wo32
    Wo_ = Wog - np.mean(Wog, axis=0, keepdims=True)
    bo_ = f(bo, np.float32) + bo_n @ Wo32

    bf = ml_dtypes.bfloat16
    wbg_h = np.concatenate([Wb_, Wgb_], axis=1).astype(bf)
    wago_h = np.concatenate([Wa_, Wga_, Wgo_], axis=1).astype(bf)
    wo_h = Wo_.astype(bf)
    bbg_h = np.concatenate([bb_, bgb_])[None, :].astype(np.float32)
    bago_h = np.concatenate([ba_, bga_, bgo_])[None, :].astype(np.float32)

    with_bias = bool(
        np.any(bbg_h) or np.any(bago_h) or np.any(bo_)
    )
    with_mask = not bool(np.all(mask == 1.0))

    # host-side LN stats: rstd per row of z, folded into z itself
    zf = z[0].reshape(n * n, D)
    m = zf.mean(axis=1, keepdims=True)
    v = ((zf - m) ** 2).mean(axis=1, keepdims=True)
    r = 1.0 / np.sqrt(v + EPS)
    zsf = (zf * r).astype(np.float32)

    SH = n // W
    NT = SH * n // P
    in_maps = []
    for mi in range(W):
        rows = zsf[SH * n * mi : SH * n * (mi + 1)]  # [SH*n, D]
        # transpose to [D, rows], tile layout [P, NT*P] with rows grouped
        # in 128-row tiles: column index = t*128 + p_row
        zs_h = np.ascontiguousarray(rows.T).astype(bf)  # [D, SH*n]
        im = {
            "zs": zs_h,
            "wbg": wbg_h,
            "wago": wago_h,
            "wo": wo_h,
        }
        if with_bias:
            im["bbg"] = bbg_h
            im["bago"] = bago_h
            im["bo"] = bo_[None, :].astype(np.float32)
        if with_mask:
            msk = mask[0].reshape(n * n)[SH * n * mi : SH * n * (mi + 1)]
            # mask per row, laid out [P, NT]: mask_sb[p, t] = mask[t*128+p]
            im["mask_sh"] = np.ascontiguousarray(
                msk.reshape(NT, P).T
            ).astype(np.float32)
        in_maps.append(im)
    return in_maps, with_bias, with_mask


def unshard(results, n=N_FULL):
    """results: list of per-core out_sh arrays [P, NT, D] -> [1, n, n, D]."""
    SH = n // W
    NT = SH * n // P
    parts = []
    for mi in range(W):
        o = results[mi].reshape(P, NT, D)
        # rows r = t*128 + p  ->  [NT, P, D] -> [SH*n, D]
        parts.append(o.transpose(1, 0, 2).reshape(SH, n, D))
    return np.concatenate(parts, axis=0)[None]


def kernel(**inputs):
    n = inputs["z"].shape[1]
    in_maps, with_bias, with_mask = prepare_host(**inputs, n=n)
    nc = _get_nc(n, with_bias, with_mask)
    res = run_bass_kernel_spmd(nc, in_maps, list(range(W)))
    out = unshard([res.results[m]["out_sh"] for m in range(W)], n=n)
    return out.astype(np.float32)


# revision 20
# speedup vs baseline: 1.8700x; 1.0396x over previous
"""Trainium2 Bass kernel for CustomTriangleMultiplicationOutgoing.

Reference computation (B=1, N=384, D=C=128):
    z_norm = LN(z) * g + b                        # over D
    left   = (z_norm@Wa + ba) * sigmoid(z_norm@Wga + bga) * mask
    right  = (z_norm@Wb + bb) * sigmoid(z_norm@Wgb + bgb) * mask
    z_out[i,j,c] = sum_k left[i,k,c] * right[j,k,c]
    z_out  = LN(z_out) * g_out + b_out            # over C
    out    = (z_out@Wo + bo) * sigmoid(z_norm@Wgo + bgo)

Key identity: row-wise LN commutes with the projection,
    LN(z) @ (g .* W) = (z * rstd) @ ((I - 11^T/D)(g .* W))
so the host passes zs = (z * rstd)^T in bf16 and centered/affine-folded
weights; the device does plain matmuls with NO LN work in phase 1.
Phase 3's LN over C uses the same centering on Wo; its mean comes from a
vector-accumulated column sum during phase 2, its mean-square from
partial-K matmuls against a ones vector fused into each phase-2 chunk.

Sharding: 1D over the first N (i) axis, 48 rows per core.  Pass A
computes only the gated right projection so the 4 c-chunked AllGathers
(bf16) start early; pass B (left + out-gate) and phase 2 (einsum, k on
partitions) hide under the collectives.  Pass A/B element-wise ops are
batched over pairs of row tiles to amortize per-instruction overhead.
"""

import numpy as np
import ml_dtypes

import concourse.bass as bass
import concourse.mybir as mybir
import concourse.tile as tile
from concourse import bacc
from concourse.masks import make_identity
from concourse.bass_utils import run_bass_kernel_spmd

F32 = mybir.dt.float32
BF16 = mybir.dt.bfloat16
EPS = 1e-5

B = 1
N_FULL = 384
D = 128
C = 128
W = 8  # cores
P = 128


def bcast_part(ap, parts):
    """Broadcast a [1, ...] AP across `parts` partitions (partition step 0)."""
    return bass.AP(tensor=ap.tensor, offset=ap.offset, ap=[[0, parts]] + ap.ap[1:])


def pair_ap(ap0, ap1):
    """Fuse two same-shape/stride APs into one with a [delta, 2] middle dim."""
    assert ap0.ap == ap1.ap and ap0.tensor is ap1.tensor
    delta = ap1.offset - ap0.offset
    return bass.AP(
        tensor=ap0.tensor, offset=ap0.offset,
        ap=[ap0.ap[0]] + [[delta, 2]] + ap0.ap[1:],
    )


def build_nc(n=N_FULL, with_bias=False, with_mask=False, nq=4):
    """Build the SPMD Bass program (same program on all 8 cores)."""
    assert n % P == 0 and n % W == 0
    SH = n // W          # rows of i per core
    KC = n // P          # 128-wide chunks of k
    NT = SH * n // P     # 128-row tiles per core (= SH*KC)
    CQ = C // nq         # c per AllGather chunk
    CQH = CQ // 2        # c per phase-2 half-load
    QP = CQ              # partitions per chunk in zt_all (c-sharded rows)

    nc = bacc.Bacc(None, num_devices=W)

    zs = nc.declare_dram_parameter("zs", [P, NT * P], BF16, isOutput=False)
    wbg = nc.declare_dram_parameter("wbg", [D, 2 * C], BF16, isOutput=False)
    wago = nc.declare_dram_parameter("wago", [D, 2 * C + D], BF16, isOutput=False)
    wo = nc.declare_dram_parameter("wo", [C, D], BF16, isOutput=False)
    if with_bias:
        bbg_p = nc.declare_dram_parameter("bbg", [1, 2 * C], F32, isOutput=False)
        bago_p = nc.declare_dram_parameter("bago", [1, 2 * C + D], F32, isOutput=False)
        bo_p = nc.declare_dram_parameter("bo", [1, D], F32, isOutput=False)
    if with_mask:
        mask_sh = nc.declare_dram_parameter("mask_sh", [P, NT], F32, isOutput=False)
    out_sh = nc.declare_dram_parameter("out_sh", [P, NT, D], F32, isOutput=True)

    # internal DRAM
    right_q = [nc.dram_tensor(f"right_{q}", [P, KC, CQ, SH], BF16) for q in range(nq)]
    gath_q = [
        nc.dram_tensor(f"gath_{q}", [W, P, KC, CQ, SH], BF16, addr_space="Shared")
        for q in range(nq)
    ]
    zout = nc.dram_tensor("zout", [C, SH * n], BF16)  # c-major einsum result

    with tile.TileContext(nc) as tc:
        with tc.tile_pool(name="singles", bufs=1) as singles:
            wbg_sb = singles.tile([D, 2 * C], BF16)
            nc.sync.dma_start(wbg_sb, wbg[:])
            wago_sb = singles.tile([D, 2 * C + D], BF16)
            nc.sync.dma_start(wago_sb, wago[:])
            wo_sb = singles.tile([C, D], BF16)
            nc.sync.dma_start(wo_sb, wo[:])
            ones_bf = singles.tile([P, 1], BF16)
            nc.vector.memset(ones_bf, 1.0)
            eps_sb = singles.tile([P, 1], F32)
            nc.vector.memset(eps_sb, EPS)
            ident = singles.tile([P, P], F32)
            make_identity(nc, ident)
            if with_bias:
                bbg_sb = singles.tile([P, 2 * C], F32)
                nc.sync.dma_start(bbg_sb, bcast_part(bbg_p[:], P))
                bago_sb = singles.tile([P, 2 * C + D], F32)
                nc.sync.dma_start(bago_sb, bcast_part(bago_p[:], P))
                bo_sb = singles.tile([P, D], F32)
                nc.sync.dma_start(bo_sb, bcast_part(bo_p[:], P))
            if with_mask:
                mask_sb = singles.tile([P, NT], F32)
                nc.sync.dma_start(mask_sb, mask_sh[:])

            # persistent stores
            gg_sb = singles.tile([P, NT, D], BF16)      # out-gate per row tile
            zt_all = singles.tile([C, NT * P], BF16)    # z_out, c on partitions
            S_acc = singles.tile([SH, n], F32)          # sum_c z_out
            nc.vector.memset(S_acc, 0.0)

            lpool = tc.alloc_tile_pool(name="lpool", bufs=1)
            L_sb = lpool.tile([P, KC, SH, C], BF16)     # left: [k, kc, i_loc, c]

            p1pool = tc.alloc_tile_pool(name="p1", bufs=1)
            zs_sb = p1pool.tile([P, NT * P], BF16)
            for ch in range(4):
                w4 = NT * P // 4
                nc.sync.dma_start(
                    zs_sb[:, ch * w4 : (ch + 1) * w4],
                    zs[:, ch * w4 : (ch + 1) * w4],
                )
            R_stage = p1pool.tile([P, KC, C, SH], BF16)  # right: [k, kc, c, j_loc]

            NPAIR = NT // 2

            # ---------------- pass A: right projection ----------------
            with (
                tc.tile_pool(name="pA_tmp", bufs=4) as tmpsA,
                tc.tile_pool(name="pA_psum", bufs=4, space="PSUM") as psumA,
            ):
                for pi in range(NPAIR):
                    t0, t1 = 2 * pi, 2 * pi + 1
                    ps = psumA.tile([P, 2, 2 * C], F32, tag="psA")
                    for j, t in enumerate((t0, t1)):
                        nc.tensor.matmul(
                            ps[:, j, :], lhsT=zs_sb[:, t * P : (t + 1) * P],
                            rhs=wbg_sb, start=True, stop=True,
                        )
                    if with_bias:
                        for j in range(2):
                            nc.vector.tensor_tensor(
                                ps[:, j, :], ps[:, j, :], bbg_sb,
                                mybir.AluOpType.add,
                            )
                    sgb = tmpsA.tile([P, 2, C], F32, tag="sgb")
                    nc.scalar.activation(
                        sgb, ps[:, :, C : 2 * C],
                        mybir.ActivationFunctionType.Sigmoid,
                    )
                    if with_mask:
                        for j, t in enumerate((t0, t1)):
                            nc.gpsimd.tensor_scalar_mul(
                                sgb[:, j, :], sgb[:, j, :], mask_sb[:, t : t + 1]
                            )
                    rout = pair_ap(
                        R_stage[:, t0 % KC, :, t0 // KC],
                        R_stage[:, t1 % KC, :, t1 // KC],
                    )
                    nc.vector.tensor_tensor(
                        rout, ps[:, :, 0:C], sgb, mybir.AluOpType.mult
                    )
                for q in range(nq):
                    nc.sync.dma_start(
                        right_q[q][:], R_stage[:, :, q * CQ : (q + 1) * CQ, :]
                    )

            # ---------------- AllGather right (c-chunked) ----------------
            for q in range(nq):
                nc.gpsimd.collective_compute(
                    "AllGather",
                    mybir.AluOpType.bypass,
                    replica_groups=[list(range(W))],
                    ins=[right_q[q][:]],
                    outs=[gath_q[q][:]],
                )

            # ---------------- pass B: left + out-gate ----------------
            with (
                tc.tile_pool(name="pB_tmp", bufs=4) as tmpsB,
                tc.tile_pool(name="pB_psum", bufs=4, space="PSUM") as psumB,
                tc.tile_pool(name="pG_psum", bufs=4, space="PSUM") as psumG,
            ):
                for pi in range(NPAIR):
                    t0, t1 = 2 * pi, 2 * pi + 1
                    ps = psumB.tile([P, 2, 2 * C], F32, tag="psB")
                    pg = psumG.tile([P, 2, D], F32, tag="psG")
                    for j, t in enumerate((t0, t1)):
                        nc.tensor.matmul(
                            ps[:, j, :], lhsT=zs_sb[:, t * P : (t + 1) * P],
                            rhs=wago_sb[:, 0 : 2 * C], start=True, stop=True,
                        )
                        nc.tensor.matmul(
                            pg[:, j, :], lhsT=zs_sb[:, t * P : (t + 1) * P],
                            rhs=wago_sb[:, 2 * C :], start=True, stop=True,
                        )
                    if with_bias:
                        for j in range(2):
                            nc.vector.tensor_tensor(
                                ps[:, j, :], ps[:, j, :], bago_sb[:, 0 : 2 * C],
                                mybir.AluOpType.add,
                            )
                            nc.vector.tensor_tensor(
                                pg[:, j, :], pg[:, j, :], bago_sb[:, 2 * C :],
                                mybir.AluOpType.add,
                            )
                    sga = tmpsB.tile([P, 2, C], F32, tag="sga")
                    nc.scalar.activation(
                        sga, ps[:, :, C : 2 * C],
                        mybir.ActivationFunctionType.Sigmoid,
                    )
                    nc.scalar.activation(
                        gg_sb[:, t0 : t0 + 2, :], pg,
                        mybir.ActivationFunctionType.Sigmoid,
                    )
                    if with_mask:
                        for j, t in enumerate((t0, t1)):
                            nc.gpsimd.tensor_scalar_mul(
                                sga[:, j, :], sga[:, j, :], mask_sb[:, t : t + 1]
                            )
                    lout = pair_ap(
                        L_sb[:, t0 % KC, t0 // KC, :],
                        L_sb[:, t1 % KC, t1 // KC, :],
                    )
                    nc.vector.tensor_tensor(
                        lout, ps[:, :, 0:C], sga, mybir.AluOpType.mult
                    )

            p1pool.release()  # zs, R_stage freed

            # ---------------- phase 2: einsum + fused z_out stats ----------------
            sqpool = tc.alloc_tile_pool(name="sq_psum", bufs=1, space="PSUM")
            # one column block per chunk-pair, self-contained matmul chains
            sq_ps = sqpool.tile([P, 2, NT], F32)  # sum_c z_out^2 per row tile
            with (
                tc.tile_pool(name="p2_r", bufs=2) as rpool,
                tc.tile_pool(name="p2_st", bufs=3) as stpool,
                tc.tile_pool(name="p2_sq", bufs=4) as sqtmp,
                tc.tile_pool(name="p2_psum", bufs=6, space="PSUM") as psum2,
            ):
                for q in range(nq):
                    for h in range(2):
                        Rh = rpool.tile([P, KC, W, CQH, SH], BF16, tag="rh")
                        for m in range(W):
                            nc.sync.dma_start(
                                Rh[:, :, m],
                                gath_q[q][m, :, :, h * CQH : (h + 1) * CQH, :],
                            )
                        for c4 in range(CQH // 4):
                            stb = stpool.tile([SH, 4, n], BF16, tag="stb")
                            for c_ in range(4):
                                cl = c4 * 4 + c_
                                c_glob = q * CQ + h * CQH + cl
                                ps = psum2.tile([SH, n], F32, tag="ps")
                                for kc in range(KC):
                                    nc.tensor.matmul(
                                        ps,
                                        lhsT=L_sb[:, kc, :, c_glob],
                                        rhs=Rh[:, kc, :, cl, :],
                                        start=(kc == 0),
                                        stop=(kc == KC - 1),
                                    )
                                if c_ % 2 == 0:
                                    nc.vector.tensor_copy(stb[:, c_, :], ps)
                                else:
                                    nc.scalar.copy(stb[:, c_, :], ps)
                                nc.vector.tensor_tensor(
                                    S_acc, S_acc, ps, mybir.AluOpType.add
                                )
                            c0 = q * CQ + h * CQH + c4 * 4
                            nc.sync.dma_start(
                                zout[c0 : c0 + 4].rearrange(
                                    "c (i j) -> i c j", i=SH
                                ),
                                stb,
                            )
                    # z_out c-rows for this chunk are final: fetch to SBUF
                    nc.sync.dma_start(
                        zt_all[q * QP : (q + 1) * QP, :],
                        zout[q * QP : (q + 1) * QP, :],
                    )
                    # fused partial mean-square over pairs of chunks
                    # (matmul base partition must be 0/32/64)
                    if q % 2 == 1:
                        lo = (q - 1) * QP
                        for t in range(NT):
                            zq = zt_all[lo : lo + 2 * QP, t * P : (t + 1) * P]
                            sqv = sqtmp.tile([P, P], BF16, tag="sqv")
                            sqs = sqv[lo : lo + 2 * QP, :]
                            nc.vector.tensor_tensor(
                                sqs, zq, zq, mybir.AluOpType.mult
                            )
                            nc.tensor.matmul(
                                sq_ps[:, q // 2, t : t + 1], lhsT=sqs,
                                rhs=ones_bf[lo : lo + 2 * QP, :],
                                start=True, stop=True,
                            )

            SQm = singles.tile([P, NT], F32)   # sum_c z_out^2, tile-major
            nc.vector.tensor_copy(SQm, sq_ps[:, 0, :])
            nc.vector.tensor_tensor(
                SQm, SQm, sq_ps[:, 1, :], mybir.AluOpType.add
            )
            sqpool.release()
            lpool.release()  # L_sb freed

            # ---------------- phase 3: LN(z_out) @ Wo * gate ----------------
            with (
                tc.tile_pool(name="p3_tmp", bufs=4) as t3,
                tc.tile_pool(name="p3_big", bufs=1) as big3,
                tc.tile_pool(name="p3_psum", bufs=4, space="PSUM") as psum3,
                tc.tile_pool(name="p3_tps", bufs=3, space="PSUM") as tpsum,
            ):
                # transpose S_acc [SH, n] -> Smat [P, NT] (tile-major stats)
                Smat = big3.tile([P, NT], F32)
                for jc in range(KC):
                    tp = tpsum.tile([P, SH], F32, tag="tp")
                    nc.tensor.transpose(
                        tp, S_acc[:, jc * P : (jc + 1) * P], ident[0:SH, 0:SH]
                    )
                    nc.vector.tensor_copy(
                        Smat[:].rearrange("p (i k) -> p k i", k=KC)[:, jc, :], tp
                    )
                mean = big3.tile([P, NT], F32)
                nc.vector.tensor_scalar_mul(mean, Smat, 1.0 / C)
                msq = big3.tile([P, NT], F32)
                nc.vector.tensor_scalar_mul(msq, SQm, 1.0 / C)
                var = big3.tile([P, NT], F32)
                nc.vector.tensor_tensor(var, mean, mean, mybir.AluOpType.mult)
                nc.vector.tensor_tensor(var, msq, var, mybir.AluOpType.subtract)
                rstd = big3.tile([P, NT], F32)
                nc.scalar.activation(
                    rstd, var, mybir.ActivationFunctionType.Sqrt, bias=eps_sb
                )
                nc.vector.reciprocal(rstd, rstd)

                ot_sb = big3.tile([P, NT, D], F32)
                for t in range(NT):
                    pr = psum3.tile([P, D], F32, tag="pr")
                    nc.tensor.matmul(
                        pr, lhsT=zt_all[:, t * P : (t + 1) * P], rhs=wo_sb,
                        start=True, stop=True,
                    )
                    if with_bias:
                        po = t3.tile([P, D], F32, tag="po")
                        nc.vector.tensor_scalar_mul(po, pr, rstd[:, t : t + 1])
                        nc.vector.tensor_tensor(po, po, bo_sb, mybir.AluOpType.add)
                        nc.vector.tensor_tensor(
                            ot_sb[:, t, :], po, gg_sb[:, t, :], mybir.AluOpType.mult
                        )
                    else:
                        nc.vector.scalar_tensor_tensor(
                            ot_sb[:, t, :], pr, rstd[:, t : t + 1],
                            gg_sb[:, t, :],
                            mybir.AluOpType.mult, mybir.AluOpType.mult,
                        )
                for ch in range(4):
                    t0 = NT // 4 * ch
                    t1 = NT // 4 * (ch + 1)
                    nc.sync.dma_start(
                        out_sh[:, t0:t1, :], ot_sb[:, t0:t1, :]
                    )

    nc.compile()
    return nc


_CACHE = {}


def _get_nc(n, with_bias, with_mask):
    key = (n, with_bias, with_mask)
    if key not in _CACHE:
        _CACHE[key] = build_nc(n=n, with_bias=with_bias, with_mask=with_mask)
    return _CACHE[key]


def prepare_host(z, mask, norm_g, norm_b, norm_out_g, norm_out_b,
                 Wa, ba, Wb, bb, Wga, bga, Wgb, bgb, Wo, bo, Wgo, bgo, n=N_FULL):
    """Fold norm affines + centering into weights; pre-normalize z rows."""
    f = np.asarray
    z = f(z, dtype=np.float32)
    mask = f(mask, dtype=np.float32)
    g = f(norm_g, np.float32)
    b = f(norm_b, np.float32)
    go = f(norm_out_g, np.float32)
    bo_n = f(norm_out_b, np.float32)

    # LN(z) @ W_aff + bias = (z*rstd) @ Wcen + (b @ W + bias),
    # Wcen = (I - J/D)(g .* W)
    def fold(Wm, bias):
        Wm = f(Wm, np.float32)
        Wg = g[:, None] * Wm
        Wcen = Wg - np.mean(Wg, axis=0, keepdims=True)
        return Wcen, f(bias, np.float32) + b @ Wm

    Wa_, ba_ = fold(Wa, ba)
    Wga_, bga_ = fold(Wga, bga)
    Wb_, bb_ = fold(Wb, bb)
    Wgb_, bgb_ = fold(Wgb, bgb)
    Wgo_, bgo_ = fold(Wgo, bgo)
    Wo32 = f(Wo, np.float32)
    Wog = go[:, None] * Wo32
    Wo_ = Wog - np.mean(Wog, axis=0, keepdims=True)
    bo_ = f(bo, np.float32) + bo_n @ Wo32

    bf = ml_dtypes.bfloat16
    wbg_h = np.concatenate([Wb_, Wgb_], axis=1).astype(bf)
    wago_h = np.concatenate([Wa_, Wga_, Wgo_], axis=1).astype(bf)
    wo_h = Wo_.astype(bf)
    bbg_h = np.concatenate([bb_, bgb_])[None, :].astype(np.float32)
    bago_h = np.concatenate([ba_, bga_, bgo_])[None, :].astype(np.float32)

    with_bias = bool(np.any(bbg_h) or np.any(bago_h) or np.any(bo_))
    with_mask = not bool(np.all(mask == 1.0))

    # host-side LN stats: rstd per row of z, folded into z itself
    zf = z[0].reshape(n * n, D)
    m = zf.mean(axis=1, keepdims=True)
    v = ((zf - m) ** 2).mean(axis=1, keepdims=True)
    r = 1.0 / np.sqrt(v + EPS)
    zsf = (zf * r).astype(np.float32)

    SH = n // W
    NT = SH * n // P
    in_maps = []
    for mi in range(W):
        rows = zsf[SH * n * mi : SH * n * (mi + 1)]  # [SH*n, D]
        zs_h = np.ascontiguousarray(rows.T).astype(bf)  # [D, SH*n]
        im = {
            "zs": zs_h,
            "wbg": wbg_h,
            "wago": wago_h,
            "wo": wo_h,
        }
        if with_bias:
            im["bbg"] = bbg_h
            im["bago"] = bago_h
            im["bo"] = bo_[None, :].astype(np.float32)
        if with_mask:
            msk = mask[0].reshape(n * n)[SH * n * mi : SH * n * (mi + 1)]
            im["mask_sh"] = np.ascontiguousarray(
                msk.reshape(NT, P).T
            ).astype(np.float32)
        in_maps.append(im)
    return in_maps, with_bias, with_mask


def unshard(results, n=N_FULL):
    """results: list of per-core out_sh arrays [P, NT, D] -> [1, n, n, D]."""
    SH = n // W
    NT = SH * n // P
    parts = []
    for mi in range(W):
        o = results[mi].reshape(P, NT, D)
        parts.append(o.transpose(1, 0, 2).reshape(SH, n, D))
    return np.concatenate(parts, axis=0)[None]


def kernel(**inputs):
    n = inputs["z"].shape[1]
    in_maps, with_bias, with_mask = prepare_host(**inputs, n=n)
    nc = _get_nc(n, with_bias, with_mask)
    res = run_bass_kernel_spmd(nc, in_maps, list(range(W)))
    out = unshard([res.results[m]["out_sh"] for m in range(W)], n=n)
    return out.astype(np.float32)
